# revision 1
# baseline (speedup 1.0000x reference)
"""GRU-D decoder kernel for Trainium2 (8 NeuronCores, data-parallel over batch).

Math (mask == ones everywhere, which the reference hardcodes):
  x_hat = C (constant), d = dt broadcast, gamma_x unused.
  gamma[t,b,j] = exp(-relu(dt[t,b] * colsum(Wgh)[j] + bgh[j]))   (precomputed host-side)
  per step: hdec = gamma_t * h
            z = sigmoid(hdec @ Wz_h + Az0);  r = sigmoid(hdec @ Wr_h + Ar0)
            htl = tanh((r*hdec) @ Wh_h + Ah0)
            h = hdec + z*(htl - hdec)
  out[t] = h_t @ Wlin + blin
  where A?0 = C @ W?_x + colsum(W?_m) + b?  (time-constant, precomputed host-side).

Device layout: everything transposed (H on partitions as 4 tiles of 128,
batch=64 on the free dim), packed as SBUF tiles (128, 4*64) with column
index = kt*64 + b.  Gate matmuls use the weight blocks as stationary
operands and hdec slices as moving operands; outputs land natively in the
same transposed layout, so no transposes are needed anywhere.  The
per-step tail (tanh/blend/decay) is split into two column halves so the
tensor engine can start the next group while the tail of the previous
half is still on Scalar/Vector.
"""

import numpy as np
import ml_dtypes

T, B, H, O = 100, 512, 512, 512
NCORES = 8
BL = B // NCORES  # 64
KC = 4  # contraction chunks of 128
JT = 4  # output j-tiles of 128
FR = JT * BL  # 256
HB = FR // 2  # 128 (half of the free dim; = 2 j-tiles)
GCH = 20  # gamma chunk (steps per DMA)

_BUILD_CACHE = {}


def _build_program():
    if "nc" in _BUILD_CACHE:
        return _BUILD_CACHE["nc"]

    import concourse.tile as tile
    import concourse.mybir as mybir
    from concourse import bacc
    from contextlib import ExitStack

    f32 = mybir.dt.float32
    bf16 = mybir.dt.bfloat16
    AF = mybir.ActivationFunctionType

    nc = bacc.Bacc("TRN2", target_bir_lowering=False, debug=False,
                   num_devices=NCORES)

    gam_d = nc.dram_tensor("gam", [128, T, FR], f32, kind="ExternalInput")
    wzr_d = nc.dram_tensor("wzr", [128, KC * 2 * JT * 128], bf16, kind="ExternalInput")
    wht_d = nc.dram_tensor("wht", [128, KC * JT * 128], bf16, kind="ExternalInput")
    wlin_d = nc.dram_tensor("wlin", [128, KC * O], bf16, kind="ExternalInput")
    a0z_d = nc.dram_tensor("a0z", [128, FR], bf16, kind="ExternalInput")
    a0r_d = nc.dram_tensor("a0r", [128, FR], bf16, kind="ExternalInput")
    a0h_d = nc.dram_tensor("a0h", [128, FR], bf16, kind="ExternalInput")
    ident_d = nc.dram_tensor("ident", [128, 128], bf16, kind="ExternalInput")
    ones_d = nc.dram_tensor("ones64", [1, BL], bf16, kind="ExternalInput")
    blinr_d = nc.dram_tensor("blinr", [1, O], bf16, kind="ExternalInput")
    out_d = nc.dram_tensor("out", [T, BL, O], f32, kind="ExternalOutput")

    with tile.TileContext(nc) as tc, ExitStack() as ctx:
        constp = ctx.enter_context(tc.tile_pool(name="const", bufs=1))
        gpool = ctx.enter_context(tc.tile_pool(name="gam", bufs=2))
        statep = ctx.enter_context(tc.tile_pool(name="state", bufs=1))
        hdp = ctx.enter_context(tc.tile_pool(name="hd", bufs=2))
        actp = ctx.enter_context(tc.tile_pool(name="act", bufs=2))
        pzp = ctx.enter_context(tc.tile_pool(name="pz", bufs=1, space="PSUM"))
        prp = ctx.enter_context(tc.tile_pool(name="pr", bufs=1, space="PSUM"))
        php0 = ctx.enter_context(tc.tile_pool(name="ph0", bufs=1, space="PSUM"))
        php1 = ctx.enter_context(tc.tile_pool(name="ph1", bufs=1, space="PSUM"))
        pjp = ctx.enter_context(tc.tile_pool(name="pj", bufs=2, space="PSUM"))

        wzr = constp.tile([128, KC * 2 * JT * 128], bf16)
        nc.sync.dma_start(wzr[:], wzr_d[:])
        wht = constp.tile([128, KC * JT * 128], bf16)
        nc.sync.dma_start(wht[:], wht_d[:])
        wlin = constp.tile([128, KC * O], bf16)
        nc.sync.dma_start(wlin[:], wlin_d[:])
        a0z = constp.tile([128, FR], bf16)
        nc.sync.dma_start(a0z[:], a0z_d[:])
        a0r = constp.tile([128, FR], bf16)
        nc.sync.dma_start(a0r[:], a0r_d[:])
        a0h = constp.tile([128, FR], bf16)
        nc.sync.dma_start(a0h[:], a0h_d[:])
        ident = constp.tile([128, 128], bf16)
        nc.sync.dma_start(ident[:], ident_d[:])
        ones64 = constp.tile([1, BL], bf16)
        nc.sync.dma_start(ones64[:], ones_d[:])
        blinr = constp.tile([1, O], bf16)
        nc.sync.dma_start(blinr[:], blinr_d[:])

        h = statep.tile([128, FR], f32)
        nc.vector.memset(h[:], 0.0)

        def wzr_blk(g, jo, kc):
            i = ((kc * 2 + g) * JT + jo) * 128
            return wzr[:, i:i + 128]

        def wht_blk(jo, kc):
            i = (kc * JT + jo) * 128
            return wht[:, i:i + 128]

        # gamma chunks, preloaded half a chunk ahead
        chunks = {}

        def ensure_chunk(c):
            if c in chunks or c * GCH >= T:
                return
            t0 = c * GCH
            t1 = min(t0 + GCH, T)
            gt = gpool.tile([128, GCH * FR], f32, tag="gchunk")
            nc.sync.dma_start(gt[:, 0:(t1 - t0) * FR], gam_d[:, t0:t1, :])
            chunks[c] = gt

        def gamma_half(tt, hf):
            c2, o2 = divmod(tt, GCH)
            return chunks[c2][:, o2 * FR + hf * HB: o2 * FR + (hf + 1) * HB]

        ensure_chunk(0)

        # step-0 decayed state is zero
        hdf = hdp.tile([128, FR], f32, tag="hdf")
        nc.vector.memset(hdf[:], 0.0)
        hdb = hdp.tile([128, FR], bf16, tag="hdb")
        nc.vector.memset(hdb[:], 0.0)

        hbf_prev = None
        pj_prev = None

        for t in range(T):
            c, o = divmod(t, GCH)
            if o == GCH // 2:
                ensure_chunk(c + 1)

            # ---- output DMA for step t-1 (projection ran at the end of t-1)
            if pj_prev is not None:
                osb = actp.tile([BL, O], f32, tag="osb")
                nc.scalar.copy(osb[:], pj_prev[:])
                nc.sync.dma_start(out_d[t - 1], osb[:])

            # ---- r gate matmuls, jo-major: each pr j-slice completes after 4
            # MMs so sigmoid(r) halves start while later slices still run
            pr = prp.tile([128, FR], f32, tag="pr")
            nc.tensor.matmul(pr[:], ident[:], a0r[:], start=True, stop=False)
            for jo in range(JT):
                for kc in range(KC):
                    nc.tensor.matmul(
                        pr[:, jo * BL:(jo + 1) * BL],
                        wzr_blk(1, jo, kc),
                        hdb[:, kc * BL:(kc + 1) * BL],
                        start=False, stop=(kc == KC - 1),
                    )
            rb = actp.tile([128, FR], bf16, tag="rb")
            nc.scalar.activation(rb[:, 0:HB], pr[:, 0:HB], AF.Sigmoid)
            nc.scalar.activation(rb[:, HB:FR], pr[:, HB:FR], AF.Sigmoid)
            rh = hdp.tile([128, FR], bf16, tag="rh")
            nc.vector.tensor_mul(rh[:, 0:HB], rb[:, 0:HB], hdb[:, 0:HB])
            nc.vector.tensor_mul(rh[:, HB:FR], rb[:, HB:FR], hdb[:, HB:FR])

            # ---- z gate first half (jo 0,1)
            pz = pzp.tile([128, FR], f32, tag="pz")
            nc.tensor.matmul(pz[:], ident[:], a0z[:], start=True, stop=False)
            for jo in (0, 1):
                for kc in range(KC):
                    nc.tensor.matmul(
                        pz[:, jo * BL:(jo + 1) * BL],
                        wzr_blk(0, jo, kc),
                        hdb[:, kc * BL:(kc + 1) * BL],
                        start=False, stop=(kc == KC - 1),
                    )

            # ---- candidate gate, kc-chunks 0,1 (gated only by rh half 0)
            ph0 = php0.tile([128, HB], f32, tag="ph0")
            ph1 = php1.tile([128, HB], f32, tag="ph1")
            nc.tensor.matmul(ph0[:], ident[:], a0h[:, 0:HB], start=True, stop=False)
            nc.tensor.matmul(ph1[:], ident[:], a0h[:, HB:FR], start=True, stop=False)
            for kc in (0, 1):
                for jo in range(JT):
                    tgt = ph0 if jo < 2 else ph1
                    nc.tensor.matmul(
                        tgt[:, (jo % 2) * BL:(jo % 2 + 1) * BL],
                        wht_blk(jo, kc),
                        rh[:, kc * BL:(kc + 1) * BL],
                        start=False, stop=False,
                    )

            # ---- z gate second half (jo 2,3)
            for jo in (2, 3):
                for kc in range(KC):
                    nc.tensor.matmul(
                        pz[:, jo * BL:(jo + 1) * BL],
                        wzr_blk(0, jo, kc),
                        hdb[:, kc * BL:(kc + 1) * BL],
                        start=False, stop=(kc == KC - 1),
                    )
            zf = actp.tile([128, FR], f32, tag="zf")
            nc.scalar.activation(zf[:, 0:HB], pz[:, 0:HB], AF.Sigmoid)
            nc.scalar.activation(zf[:, HB:FR], pz[:, HB:FR], AF.Sigmoid)

            # ---- candidate gate, kc-chunks 2,3; jo 0,1 slices finish first so
            # tanh(half 0) can start while jo 2,3 still accumulate
            for jo in (0, 1):
                nc.tensor.matmul(
                    ph0[:, jo * BL:(jo + 1) * BL], wht_blk(jo, 2),
                    rh[:, 2 * BL:3 * BL], start=False, stop=False)
                nc.tensor.matmul(
                    ph0[:, jo * BL:(jo + 1) * BL], wht_blk(jo, 3),
                    rh[:, 3 * BL:4 * BL], start=False, stop=True)
            for jo in (2, 3):
                nc.tensor.matmul(
                    ph1[:, (jo - 2) * BL:(jo - 1) * BL], wht_blk(jo, 2),
                    rh[:, 2 * BL:3 * BL], start=False, stop=False)
                nc.tensor.matmul(
                    ph1[:, (jo - 2) * BL:(jo - 1) * BL], wht_blk(jo, 3),
                    rh[:, 3 * BL:4 * BL], start=False, stop=True)

            # ---- blend: h = (1-z)*hdec + z*htl, with (1-z)*hdec computed
            # off the tanh critical path
            zm = actp.tile([128, FR], f32, tag="zm")
            nc.vector.tensor_scalar(zm[:, 0:HB], zf[:, 0:HB], -1.0, 1.0,
                                    mybir.AluOpType.mult, mybir.AluOpType.add)
            pp_ = actp.tile([128, FR], f32, tag="pp")
            nc.vector.tensor_mul(pp_[:, 0:HB], zm[:, 0:HB], hdf[:, 0:HB])
            nc.vector.tensor_scalar(zm[:, HB:FR], zf[:, HB:FR], -1.0, 1.0,
                                    mybir.AluOpType.mult, mybir.AluOpType.add)
            nc.vector.tensor_mul(pp_[:, HB:FR], zm[:, HB:FR], hdf[:, HB:FR])

            hdf_n = hdb_n = None
            if t + 1 < T:
                hdf_n = hdp.tile([128, FR], f32, tag="hdf")
                hdb_n = hdp.tile([128, FR], bf16, tag="hdb")
            for hf, ph in ((0, ph0), (1, ph1)):
                sl = slice(hf * HB, (hf + 1) * HB)
                htl = actp.tile([128, HB], f32, tag=f"htl{hf}")
                nc.scalar.activation(htl[:], ph[:], AF.Tanh)
                qq = actp.tile([128, HB], f32, tag=f"qq{hf}")
                nc.vector.tensor_mul(qq[:], zf[:, sl], htl[:])
                nc.vector.tensor_add(h[:, sl], qq[:], pp_[:, sl])
                if t + 1 < T:
                    # bf16 decayed state straight from the fp32 mul (cast on write)
                    nc.vector.tensor_mul(hdb_n[:, sl], gamma_half(t + 1, hf), h[:, sl])
            if t + 1 < T:
                nc.vector.tensor_mul(hdf_n[:], chunks[(t + 1) // GCH][
                    :, ((t + 1) % GCH) * FR:((t + 1) % GCH + 1) * FR], h[:])
                hdf, hdb = hdf_n, hdb_n

            hbf = actp.tile([128, FR], bf16, tag="hbf")
            nc.scalar.copy(hbf[:], h[:])
            hbf_prev = hbf

            # ---- projection of h(t) at the end of the PE stream (fills the
            # tanh/blend tail); DMA'd out at the start of step t+1
            pj_prev = pjp.tile([BL, O], f32, tag="pj")
            nc.tensor.matmul(pj_prev[:], ones64[:], blinr[:], start=True, stop=False)
            for kc in range(KC):
                nc.tensor.matmul(
                    pj_prev[:],
                    hbf_prev[:, kc * BL:(kc + 1) * BL],
                    wlin[:, kc * O:(kc + 1) * O],
                    start=False, stop=(kc == KC - 1),
                )

        osb = actp.tile([BL, O], f32, tag="osb")
        nc.scalar.copy(osb[:], pj_prev[:])
        nc.sync.dma_start(out_d[T - 1], osb[:])

    nc.compile()
    _BUILD_CACHE["nc"] = nc
    return nc


def _host_prep(C, t, Wz, bz, Wr, br, Wh, bh, Wgh, bgh, Wlin, blin):
    """Build per-core input maps (all the precomputed, packed device tensors)."""
    bf = ml_dtypes.bfloat16

    s = Wgh.sum(axis=0)  # (H,)
    t3 = t[:, :, 0]  # (T,B)
    dt = np.concatenate([np.zeros((1, B), np.float32), t3[1:] - t3[:-1]], axis=0)
    # gamma (T,B,H) fp32
    gam = np.exp(-np.maximum(dt[:, :, None] * s[None, None, :] + bgh[None, None, :], 0.0)).astype(np.float32)

    def gate_const(W, b):
        # C @ W_x + colsum(W_m) + b  -> (B,H)
        return C @ W[0:H] + (W[2 * H:3 * H].sum(axis=0) + b)[None, :]

    Az0 = gate_const(Wz, bz).astype(np.float32)
    Ar0 = gate_const(Wr, br).astype(np.float32)
    Ah0 = gate_const(Wh, bh).astype(np.float32)

    Wg = np.stack([Wz[H:2 * H], Wr[H:2 * H]])  # (2,H,H)
    # wzr packed: [k, (kc,g,jo,m)]
    wzr = Wg.reshape(2, KC, 128, JT, 128).transpose(2, 1, 0, 3, 4).reshape(128, KC * 2 * JT * 128)
    wht = Wh[H:2 * H].reshape(KC, 128, JT, 128).transpose(1, 0, 2, 3).reshape(128, KC * JT * 128)
    wlin = Wlin.reshape(KC, 128, O).transpose(1, 0, 2).reshape(128, KC * O)
    wzr = np.ascontiguousarray(wzr, dtype=bf)
    wht = np.ascontiguousarray(wht, dtype=bf)
    wlin = np.ascontiguousarray(wlin, dtype=bf)
    ident = np.eye(128, dtype=bf)

    in_maps = []
    for i in range(NCORES):
        sl = slice(i * BL, (i + 1) * BL)
        gf = gam[:, sl, :]  # (T,BL,H)
        # gam packed: [p, t, kt*BL+b]
        gp = np.ascontiguousarray(gf.reshape(T, BL, KC, 128).transpose(3, 0, 2, 1).reshape(128, T, KC * BL))

        def packA(A):
            return np.ascontiguousarray(
                A[sl].reshape(BL, JT, 128).transpose(2, 1, 0).reshape(128, JT * BL), dtype=bf)

        in_maps.append({
            "gam": gp,
            "wzr": wzr,
            "wht": wht,
            "wlin": wlin,
            "a0z": packA(Az0),
            "a0r": packA(Ar0),
            "a0h": packA(Ah0),
            "ident": ident,
            "ones64": np.ones((1, BL), dtype=bf),
            "blinr": np.ascontiguousarray(blin.reshape(1, O), dtype=bf),
        })
    return in_maps


def kernel(C, t, mask, Wz, bz, Wr, br, Wh, bh, Wgh, bgh, wgx, bgx, Wlin, blin,
           _trace=False, _trace_kwargs=None):
    C = np.asarray(C, np.float32)
    t = np.asarray(t, np.float32)
    nc = _build_program()
    in_maps = _host_prep(C, t,
                         np.asarray(Wz, np.float32), np.asarray(bz, np.float32),
                         np.asarray(Wr, np.float32), np.asarray(br, np.float32),
                         np.asarray(Wh, np.float32), np.asarray(bh, np.float32),
                         np.asarray(Wgh, np.float32), np.asarray(bgh, np.float32),
                         np.asarray(Wlin, np.float32), np.asarray(blin, np.float32))

    from concourse.bass_utils import run_bass_kernel_spmd
    res = run_bass_kernel_spmd(nc, in_maps, list(range(NCORES)),
                               trace=_trace, **(_trace_kwargs or {}))
    outs = [res.results[i]["out"] for i in range(NCORES)]
    full = np.concatenate(outs, axis=1).astype(np.float32)  # (T,B,O)
    kernel._last_results = res
    return full



# revision 5
# speedup vs baseline: 1.6032x; 1.6032x over previous
"""GRU-D decoder kernel for Trainium2 (8 NeuronCores, data-parallel over batch).

Math (mask == ones everywhere, which the reference hardcodes):
  x_hat = C (constant), d = dt broadcast, gamma_x unused.
  gamma[t,b,j] = exp(-relu(dt[t,b] * colsum(Wgh)[j] + bgh[j]))   (precomputed host-side)
  per step: hdec = gamma_t * h
            z = sigmoid(hdec @ Wz_h + Az0);  r = sigmoid(hdec @ Wr_h + Ar0)
            htl = tanh((r*hdec) @ Wh_h + Ah0)
            h = hdec + z*(htl - hdec)
  out[t] = h_t @ Wlin            (blin added host-side after the gather)
  where A?0 = C @ W?_x + colsum(W?_m) + b?  (time-constant, precomputed host-side).

Device layout: everything transposed (H on partitions as 4 tiles of 128,
batch=64 on the free dim), packed as SBUF tiles (128, 4*64) with column
index = kt*64 + b.  All state is bf16 (validated: global rel err ~5e-3).

v2 structure (vs the v1 baseline):
  - Per-step PE stream is r(16) z(16) htl(16, jo-major) proj(4, even steps)
    next-step psum inits(4).  The projection + inits fill the tanh/blend
    tail so the PE never idles long enough for the HAM clock gate to
    re-throttle (v1 oscillated 1.2<->2.4 GHz the whole run).
  - Projection batches TWO timesteps per weight pass: lhsT = h ring slots
    (t, t+1) giving M=128, rhs = Wlin tiles at N=512.  5 MMs/step -> 2.
  - All gate activations output bf16; the h state is a bf16 ring buffer
    (4 slots) read directly as the projection's stationary operand, so the
    v1 per-step fp32 state + hbf copy + separate osb copy disappear.
  - ph0/ph1 psum pools are double-buffered so next-step inits never wait
    on the current tanh reads.
"""

import numpy as np
import ml_dtypes

T, B, H, O = 100, 512, 512, 512
NCORES = 8
BL = B // NCORES  # 64
KC = 4  # contraction chunks of 128
JT = 4  # output j-tiles of 128
FR = JT * BL  # 256
HB = FR // 2  # 128 (half of the free dim; = 2 j-tiles)
GCH = 20  # gamma chunk (steps per DMA)
PSB = 512  # psum bank width in fp32

_BUILD_CACHE = {}


def _build_program():
    if "nc" in _BUILD_CACHE:
        return _BUILD_CACHE["nc"]

    import concourse.tile as tile
    import concourse.mybir as mybir
    from concourse import bacc
    from contextlib import ExitStack

    f32 = mybir.dt.float32
    bf16 = mybir.dt.bfloat16
    AF = mybir.ActivationFunctionType

    nc = bacc.Bacc("TRN2", target_bir_lowering=False, debug=False,
                   num_devices=NCORES)

    gam_d = nc.dram_tensor("gam", [128, T, FR], bf16, kind="ExternalInput")
    wzr_d = nc.dram_tensor("wzr", [128, KC * 2 * JT * 128], bf16, kind="ExternalInput")
    wht_d = nc.dram_tensor("wht", [128, KC * JT * 128], bf16, kind="ExternalInput")
    wlin_d = nc.dram_tensor("wlin", [128, KC * O], bf16, kind="ExternalInput")
    a0z_d = nc.dram_tensor("a0z", [128, FR], bf16, kind="ExternalInput")
    a0r_d = nc.dram_tensor("a0r", [128, FR], bf16, kind="ExternalInput")
    a0h_d = nc.dram_tensor("a0h", [128, FR], bf16, kind="ExternalInput")
    ident_d = nc.dram_tensor("ident", [128, 128], bf16, kind="ExternalInput")
    out_d = nc.dram_tensor("out", [T, BL, O], f32, kind="ExternalOutput")

    with tile.TileContext(nc) as tc, ExitStack() as ctx:
        constp = ctx.enter_context(tc.tile_pool(name="const", bufs=1))
        gpool = ctx.enter_context(tc.tile_pool(name="gam", bufs=2))
        ringp = ctx.enter_context(tc.tile_pool(name="ring", bufs=1))
        hdp = ctx.enter_context(tc.tile_pool(name="hd", bufs=2))
        actp = ctx.enter_context(tc.tile_pool(name="act", bufs=2))
        osbp = ctx.enter_context(tc.tile_pool(name="osb", bufs=2))
        prp = ctx.enter_context(tc.tile_pool(name="pr", bufs=1, space="PSUM"))
        pzp = ctx.enter_context(tc.tile_pool(name="pz", bufs=1, space="PSUM"))
        php0 = ctx.enter_context(tc.tile_pool(name="ph0", bufs=2, space="PSUM"))
        php1 = ctx.enter_context(tc.tile_pool(name="ph1", bufs=2, space="PSUM"))
        pjp = ctx.enter_context(tc.tile_pool(name="pj", bufs=2, space="PSUM"))

        wzr = constp.tile([128, KC * 2 * JT * 128], bf16)
        nc.sync.dma_start(wzr[:], wzr_d[:])
        wht = constp.tile([128, KC * JT * 128], bf16)
        nc.sync.dma_start(wht[:], wht_d[:])
        wlin = constp.tile([128, KC * O], bf16)
        nc.sync.dma_start(wlin[:], wlin_d[:])
        a0z = constp.tile([128, FR], bf16)
        nc.sync.dma_start(a0z[:], a0z_d[:])
        a0r = constp.tile([128, FR], bf16)
        nc.sync.dma_start(a0r[:], a0r_d[:])
        a0h = constp.tile([128, FR], bf16)
        nc.sync.dma_start(a0h[:], a0h_d[:])
        ident = constp.tile([128, 128], bf16)
        nc.sync.dma_start(ident[:], ident_d[:])

        # h ring buffer, bf16: column = kt*(4*BL) + slot*BL + b with slot =
        # t%4, so a projection pair (t, t+1) is a contiguous 128-column slice
        # per kt block (walrus requires 2D stationary APs).
        hring = ringp.tile([128, KC * 4 * BL], bf16)

        def ring_blk(kt, s, n=1):
            return hring[:, kt * 4 * BL + s * BL: kt * 4 * BL + (s + n) * BL]

        def wzr_blk(g, jo, kc):
            i = ((kc * 2 + g) * JT + jo) * 128
            return wzr[:, i:i + 128]

        def wht_blk(jo, kc):
            i = (kc * JT + jo) * 128
            return wht[:, i:i + 128]

        # gamma chunks, preloaded half a chunk ahead
        chunks = {}

        def ensure_chunk(c):
            if c in chunks or c * GCH >= T:
                return
            t0 = c * GCH
            t1 = min(t0 + GCH, T)
            gt = gpool.tile([128, GCH * FR], bf16, tag="gchunk")
            nc.sync.dma_start(gt[:, 0:(t1 - t0) * FR], gam_d[:, t0:t1, :])
            chunks[c] = gt

        def gamma_half(tt, hf):
            c2, o2 = divmod(tt, GCH)
            return chunks[c2][:, o2 * FR + hf * HB: o2 * FR + (hf + 1) * HB]

        ensure_chunk(0)

        # step-0 decayed state is zero
        hd = hdp.tile([128, FR], bf16, tag="hd")
        nc.vector.memset(hd[:], 0.0)

        def make_inits():
            """Allocate next step's psum tiles and preload the gate constants
            (identity matmuls run at the end of the previous PE stream)."""
            pr = prp.tile([128, PSB], f32, tag="pr")
            nc.tensor.matmul(pr[:, 0:FR], ident[:], a0r[:], start=True, stop=False)
            pz = pzp.tile([128, PSB], f32, tag="pz")
            nc.tensor.matmul(pz[:, 0:FR], ident[:], a0z[:], start=True, stop=False)
            ph0 = php0.tile([128, PSB], f32, tag="ph0")
            nc.tensor.matmul(ph0[:, 0:HB], ident[:], a0h[:, 0:HB], start=True, stop=False)
            ph1 = php1.tile([128, PSB], f32, tag="ph1")
            nc.tensor.matmul(ph1[:, 0:HB], ident[:], a0h[:, HB:FR], start=True, stop=False)
            return pr, pz, ph0, ph1

        def issue_proj(t0):
            """Project the h pair (t0, t0+1) from the ring: 4 accumulating
            matmuls with M=128 (two steps x 64 batch), N=512."""
            base = t0 % 4
            pj = pjp.tile([128, PSB], f32, tag="pj")
            for kc in range(KC):
                nc.tensor.matmul(
                    pj[:],
                    ring_blk(kc, base, 2),
                    wlin[:, kc * O:(kc + 1) * O],
                    start=(kc == 0), stop=(kc == KC - 1),
                )
            return pj

        def evac_proj(t0, pj):
            osb = osbp.tile([128, O], f32, tag="osb")
            nc.scalar.copy(osb[:, 0:256], pj[:, 0:256])
            nc.vector.tensor_copy(osb[:, 256:512], pj[:, 256:512])
            nc.sync.dma_start(out_d[t0:t0 + 2], osb[:])

        pr, pz, ph0, ph1 = make_inits()
        pj_pending = None  # (t0, pj)

        for t in range(T):
            c, o = divmod(t, GCH)
            if o == GCH // 2:
                ensure_chunk(c + 1)
            slot = t % 4

            # ---- r gate matmuls
            for jo in range(JT):
                for kc in range(KC):
                    nc.tensor.matmul(
                        pr[:, jo * BL:(jo + 1) * BL],
                        wzr_blk(1, jo, kc),
                        hd[:, kc * BL:(kc + 1) * BL],
                        start=False, stop=(kc == KC - 1),
                    )
            # ---- z gate matmuls (fill the sigmoid(r)/rh window)
            for jo in range(JT):
                for kc in range(KC):
                    nc.tensor.matmul(
                        pz[:, jo * BL:(jo + 1) * BL],
                        wzr_blk(0, jo, kc),
                        hd[:, kc * BL:(kc + 1) * BL],
                        start=False, stop=(kc == KC - 1),
                    )
            rb = actp.tile([128, FR], bf16, tag="rb")
            nc.scalar.activation(rb[:], pr[:, 0:FR], AF.Sigmoid)
            rh = hdp.tile([128, FR], bf16, tag="rh")
            nc.vector.tensor_mul(rh[:], rb[:], hd[:])

            # ---- candidate gate, jo-major: ph0 (h-half 0) completes first so
            # tanh(half 0) overlaps the jo 2,3 accumulation
            for jo in range(JT):
                tgt, col = (ph0, jo) if jo < 2 else (ph1, jo - 2)
                for kc in range(KC):
                    nc.tensor.matmul(
                        tgt[:, col * BL:(col + 1) * BL],
                        wht_blk(jo, kc),
                        rh[:, kc * BL:(kc + 1) * BL],
                        start=False, stop=(kc == KC - 1),
                    )
            zf = actp.tile([128, FR], bf16, tag="zf")
            nc.scalar.activation(zf[:], pz[:, 0:FR], AF.Sigmoid)

            # ---- tail fill on PE: batched pair projection + next-step inits
            if t >= 2 and t % 2 == 0:
                pj_pending = (t - 2, issue_proj(t - 2))
            ph0_r, ph1_r = ph0, ph1
            if t + 1 < T:
                pr, pz, ph0, ph1 = make_inits()

            # ---- tanh + blend (h = hd + z*(htl-hd)), then decay for t+1
            hd_n = None
            if t + 1 < T:
                hd_n = hdp.tile([128, FR], bf16, tag="hd")
            for hf, ph in ((0, ph0_r), (1, ph1_r)):
                sl = slice(hf * HB, (hf + 1) * HB)
                htl = actp.tile([128, HB], bf16, tag=f"htl{hf}")
                nc.scalar.activation(htl[:], ph[:, 0:HB], AF.Tanh)
                dd = actp.tile([128, HB], bf16, tag=f"dd{hf}")
                nc.vector.tensor_sub(dd[:], htl[:], hd[:, sl])
                ee = actp.tile([128, HB], bf16, tag=f"ee{hf}")
                nc.vector.tensor_mul(ee[:], zf[:, sl], dd[:])
                for q in range(2):
                    kt = 2 * hf + q
                    ksl = slice(kt * BL, (kt + 1) * BL)
                    nc.vector.tensor_add(ring_blk(kt, slot), hd[:, ksl],
                                         ee[:, q * BL:(q + 1) * BL])
                    if t + 1 < T:
                        nc.vector.tensor_mul(
                            hd_n[:, ksl],
                            chunks[(t + 1) // GCH][
                                :, ((t + 1) % GCH) * FR + kt * BL:
                                   ((t + 1) % GCH) * FR + (kt + 1) * BL],
                            ring_blk(kt, slot))
            if t + 1 < T:
                hd = hd_n

            # ---- drain the pending projection pair (psum -> sbuf -> HBM)
            if pj_pending is not None:
                t0, pj = pj_pending
                evac_proj(t0, pj)
                pj_pending = None

        # final pair (T-2, T-1)
        evac_proj(T - 2, issue_proj(T - 2))

    nc.compile()
    _BUILD_CACHE["nc"] = nc
    return nc


def _host_prep(C, t, Wz, bz, Wr, br, Wh, bh, Wgh, bgh, Wlin):
    """Build per-core input maps (all the precomputed, packed device tensors)."""
    bf = ml_dtypes.bfloat16

    s = Wgh.sum(axis=0)  # (H,)
    t3 = t[:, :, 0]  # (T,B)
    dt = np.concatenate([np.zeros((1, B), np.float32), t3[1:] - t3[:-1]], axis=0)
    # gamma (T,B,H)
    gam = np.exp(-np.maximum(dt[:, :, None] * s[None, None, :] + bgh[None, None, :], 0.0)).astype(np.float32)

    def gate_const(W, b):
        # C @ W_x + colsum(W_m) + b  -> (B,H)
        return C @ W[0:H] + (W[2 * H:3 * H].sum(axis=0) + b)[None, :]

    Az0 = gate_const(Wz, bz).astype(np.float32)
    Ar0 = gate_const(Wr, br).astype(np.float32)
    Ah0 = gate_const(Wh, bh).astype(np.float32)

    Wg = np.stack([Wz[H:2 * H], Wr[H:2 * H]])  # (2,H,H)
    # wzr packed: [k, (kc,g,jo,m)]
    wzr = Wg.reshape(2, KC, 128, JT, 128).transpose(2, 1, 0, 3, 4).reshape(128, KC * 2 * JT * 128)
    wht = Wh[H:2 * H].reshape(KC, 128, JT, 128).transpose(1, 0, 2, 3).reshape(128, KC * JT * 128)
    wlin = Wlin.reshape(KC, 128, O).transpose(1, 0, 2).reshape(128, KC * O)
    wzr = np.ascontiguousarray(wzr, dtype=bf)
    wht = np.ascontiguousarray(wht, dtype=bf)
    wlin = np.ascontiguousarray(wlin, dtype=bf)
    ident = np.eye(128, dtype=bf)

    in_maps = []
    for i in range(NCORES):
        sl = slice(i * BL, (i + 1) * BL)
        gf = gam[:, sl, :]  # (T,BL,H)
        # gam packed: [p, t, kt*BL+b]
        gp = np.ascontiguousarray(
            gf.reshape(T, BL, KC, 128).transpose(3, 0, 2, 1).reshape(128, T, KC * BL),
            dtype=bf)

        def packA(A):
            return np.ascontiguousarray(
                A[sl].reshape(BL, JT, 128).transpose(2, 1, 0).reshape(128, JT * BL), dtype=bf)

        in_maps.append({
            "gam": gp,
            "wzr": wzr,
            "wht": wht,
            "wlin": wlin,
            "a0z": packA(Az0),
            "a0r": packA(Ar0),
            "a0h": packA(Ah0),
            "ident": ident,
        })
    return in_maps


def kernel(C, t, mask, Wz, bz, Wr, br, Wh, bh, Wgh, bgh, wgx, bgx, Wlin, blin,
           _trace=False, _trace_kwargs=None):
    C = np.asarray(C, np.float32)
    t = np.asarray(t, np.float32)
    nc = _build_program()
    in_maps = _host_prep(C, t,
                         np.asarray(Wz, np.float32), np.asarray(bz, np.float32),
                         np.asarray(Wr, np.float32), np.asarray(br, np.float32),
                         np.asarray(Wh, np.float32), np.asarray(bh, np.float32),
                         np.asarray(Wgh, np.float32), np.asarray(bgh, np.float32),
                         np.asarray(Wlin, np.float32))

    from concourse.bass_utils import run_bass_kernel_spmd
    res = run_bass_kernel_spmd(nc, in_maps, list(range(NCORES)),
                               trace=_trace, **(_trace_kwargs or {}))
    outs = [res.results[i]["out"] for i in range(NCORES)]
    full = np.concatenate(outs, axis=1).astype(np.float32)  # (T,B,O)
    full += np.asarray(blin, np.float32)[None, None, :]
    kernel._last_results = res
    return full


# revision 8
# speedup vs baseline: 1.7493x; 1.0912x over previous
"""GRU-D decoder kernel for Trainium2 (8 NeuronCores, data-parallel over batch).

Math (mask == ones everywhere, which the reference hardcodes):
  x_hat = C (constant), d = dt broadcast, gamma_x unused.
  gamma[t,b,j] = exp(-relu(dt[t,b] * colsum(Wgh)[j] + bgh[j]))   (precomputed host-side)
  per step: hdec = gamma_t * h
            z = sigmoid(hdec @ Wz_h + Az0);  r = sigmoid(hdec @ Wr_h + Ar0)
            htl = tanh((r*hdec) @ Wh_h + Ah0)
            h = hdec + z*(htl - hdec)
  out[t] = h_t @ Wlin            (blin added host-side after the gather)
  where A?0 = C @ W?_x + colsum(W?_m) + b?  (time-constant, precomputed host-side).

Device layout: everything transposed (H on partitions as 4 tiles of 128,
batch=64 on the free dim), packed as SBUF tiles (128, 4*64) with column
index = kt*64 + b.  All state is bf16 (validated: global rel err ~5e-3).

v2 structure (vs the v1 baseline):
  - Per-step PE stream is r(16) z(16) htl(16, jo-major) proj(4, even steps)
    next-step psum inits(4).  The projection + inits fill the tanh/blend
    tail so the PE never idles long enough for the HAM clock gate to
    re-throttle (v1 oscillated 1.2<->2.4 GHz the whole run).
  - Projection batches TWO timesteps per weight pass: lhsT = h ring slots
    (t, t+1) giving M=128, rhs = Wlin tiles at N=512.  5 MMs/step -> 2.
  - All gate activations output bf16; the h state is a bf16 ring buffer
    (4 slots) read directly as the projection's stationary operand, so the
    v1 per-step fp32 state + hbf copy + separate osb copy disappear.
  - ph0/ph1 psum pools are double-buffered so next-step inits never wait
    on the current tanh reads.
"""

import numpy as np
import ml_dtypes

T, B, H, O = 100, 512, 512, 512
NCORES = 8
BL = B // NCORES  # 64
KC = 4  # contraction chunks of 128
JT = 4  # output j-tiles of 128
FR = JT * BL  # 256
HB = FR // 2  # 128 (half of the free dim; = 2 j-tiles)
GCH = 20  # gamma chunk (steps per DMA)
PSB = 512  # psum bank width in fp32

_BUILD_CACHE = {}


def _build_program():
    if "nc" in _BUILD_CACHE:
        return _BUILD_CACHE["nc"]

    import concourse.tile as tile
    import concourse.mybir as mybir
    from concourse import bacc
    from contextlib import ExitStack

    f32 = mybir.dt.float32
    bf16 = mybir.dt.bfloat16
    AF = mybir.ActivationFunctionType

    nc = bacc.Bacc("TRN2", target_bir_lowering=False, debug=False,
                   num_devices=NCORES)

    gam_d = nc.dram_tensor("gam", [128, T, FR], bf16, kind="ExternalInput")
    wzr_d = nc.dram_tensor("wzr", [128, KC * 2 * JT * 128], bf16, kind="ExternalInput")
    wht_d = nc.dram_tensor("wht", [128, KC * JT * 128], bf16, kind="ExternalInput")
    wlin_d = nc.dram_tensor("wlin", [128, KC * O], bf16, kind="ExternalInput")
    a0z_d = nc.dram_tensor("a0z", [128, FR], bf16, kind="ExternalInput")
    a0r_d = nc.dram_tensor("a0r", [128, FR], bf16, kind="ExternalInput")
    a0h_d = nc.dram_tensor("a0h", [128, FR], bf16, kind="ExternalInput")
    ident_d = nc.dram_tensor("ident", [128, 128], bf16, kind="ExternalInput")
    out_d = nc.dram_tensor("out", [T, BL, O], f32, kind="ExternalOutput")

    with tile.TileContext(nc) as tc, ExitStack() as ctx:
        constp = ctx.enter_context(tc.tile_pool(name="const", bufs=1))
        gpool = ctx.enter_context(tc.tile_pool(name="gam", bufs=2))
        ringp = ctx.enter_context(tc.tile_pool(name="ring", bufs=1))
        hdp = ctx.enter_context(tc.tile_pool(name="hd", bufs=2))
        actp = ctx.enter_context(tc.tile_pool(name="act", bufs=2))
        osbp = ctx.enter_context(tc.tile_pool(name="osb", bufs=2))
        prp = ctx.enter_context(tc.tile_pool(name="pr", bufs=1, space="PSUM"))
        pzp = ctx.enter_context(tc.tile_pool(name="pz", bufs=1, space="PSUM"))
        php0 = ctx.enter_context(tc.tile_pool(name="ph0", bufs=2, space="PSUM"))
        php1 = ctx.enter_context(tc.tile_pool(name="ph1", bufs=2, space="PSUM"))
        pjp = ctx.enter_context(tc.tile_pool(name="pj", bufs=2, space="PSUM"))

        wzr = constp.tile([128, KC * 2 * JT * 128], bf16)
        nc.sync.dma_start(wzr[:], wzr_d[:])
        wht = constp.tile([128, KC * JT * 128], bf16)
        nc.sync.dma_start(wht[:], wht_d[:])
        wlin = constp.tile([128, KC * O], bf16)
        nc.sync.dma_start(wlin[:], wlin_d[:])
        a0z = constp.tile([128, FR], bf16)
        nc.sync.dma_start(a0z[:], a0z_d[:])
        a0r = constp.tile([128, FR], bf16)
        nc.sync.dma_start(a0r[:], a0r_d[:])
        a0h = constp.tile([128, FR], bf16)
        nc.sync.dma_start(a0h[:], a0h_d[:])
        ident = constp.tile([128, 128], bf16)
        nc.sync.dma_start(ident[:], ident_d[:])

        # h ring buffer, bf16: column = (kt, slot*BL + b) with slot = t%4, so
        # a projection pair (t, t+1) is a contiguous 128-column slice per kt
        # block (walrus requires 2D stationary APs).  Elementwise ops use 3D
        # strided views (two kt blocks per half).
        hring = ringp.tile([128, KC, 4 * BL], bf16)

        def ring_blk(kt, s, n=1):
            return hring[:, kt, s * BL:(s + n) * BL]

        def ring_half(hf, s):
            return hring[:, 2 * hf:2 * hf + 2, s * BL:(s + 1) * BL]

        def wzr_blk(g, jo, kc):
            i = ((kc * 2 + g) * JT + jo) * 128
            return wzr[:, i:i + 128]

        def wht_blk(jo, kc):
            i = (kc * JT + jo) * 128
            return wht[:, i:i + 128]

        # gamma chunks, preloaded half a chunk ahead
        chunks = {}

        def ensure_chunk(c):
            if c in chunks or c * GCH >= T:
                return
            t0 = c * GCH
            t1 = min(t0 + GCH, T)
            gt = gpool.tile([128, GCH * FR], bf16, tag="gchunk")
            nc.sync.dma_start(gt[:, 0:(t1 - t0) * FR], gam_d[:, t0:t1, :])
            chunks[c] = gt

        def gamma_half(tt, hf):
            c2, o2 = divmod(tt, GCH)
            return chunks[c2][:, o2 * FR + hf * HB: o2 * FR + (hf + 1) * HB]

        ensure_chunk(0)

        # step-0 decayed state is zero
        hd = hdp.tile([128, FR], bf16, tag="hd")
        nc.vector.memset(hd[:], 0.0)

        def make_inits():
            """Allocate next step's psum tiles and preload the gate constants
            (identity matmuls run at the end of the previous PE stream)."""
            pr = prp.tile([128, PSB], f32, tag="pr")
            nc.tensor.matmul(pr[:, 0:FR], ident[:], a0r[:], start=True, stop=False)
            pz = pzp.tile([128, PSB], f32, tag="pz")
            nc.tensor.matmul(pz[:, 0:FR], ident[:], a0z[:], start=True, stop=False)
            ph0 = php0.tile([128, PSB], f32, tag="ph0")
            nc.tensor.matmul(ph0[:, 0:HB], ident[:], a0h[:, 0:HB], start=True, stop=False)
            ph1 = php1.tile([128, PSB], f32, tag="ph1")
            nc.tensor.matmul(ph1[:, 0:HB], ident[:], a0h[:, HB:FR], start=True, stop=False)
            return pr, pz, ph0, ph1

        def issue_proj(t0, pj, kcs):
            """Project the h pair (t0, t0+1) from the ring: accumulating
            matmuls with M=128 (two steps x 64 batch), N=512.  Split across
            two scan steps (kcs=(0,1) then (2,3)) so both steps' PE tails
            get fill work."""
            base = t0 % 4
            for kc in kcs:
                nc.tensor.matmul(
                    pj[:],
                    ring_blk(kc, base, 2),
                    wlin[:, kc * O:(kc + 1) * O],
                    start=(kc == 0), stop=(kc == KC - 1),
                )

        def evac_proj(t0, pj):
            osb = osbp.tile([128, O], f32, tag="osb")
            nc.scalar.copy(osb[:, 0:256], pj[:, 0:256])
            nc.vector.tensor_copy(osb[:, 256:512], pj[:, 256:512])
            nc.sync.dma_start(out_d[t0:t0 + 2], osb[:])

        pr, pz, ph0, ph1 = make_inits()
        pj_cur = None

        for t in range(T):
            c, o = divmod(t, GCH)
            if o == GCH // 2:
                ensure_chunk(c + 1)
            slot = t % 4

            # ---- r gate matmuls, kc-outer so they start on partial hd
            for kc in range(KC):
                for jo in range(JT):
                    nc.tensor.matmul(
                        pr[:, jo * BL:(jo + 1) * BL],
                        wzr_blk(1, jo, kc),
                        hd[:, kc * BL:(kc + 1) * BL],
                        start=False, stop=(kc == KC - 1),
                    )
            # ---- z gate matmuls (fill the sigmoid(r)/rh window)
            for kc in range(KC):
                for jo in range(JT):
                    nc.tensor.matmul(
                        pz[:, jo * BL:(jo + 1) * BL],
                        wzr_blk(0, jo, kc),
                        hd[:, kc * BL:(kc + 1) * BL],
                        start=False, stop=(kc == KC - 1),
                    )
            rb = actp.tile([128, FR], bf16, tag="rb")
            nc.scalar.activation(rb[:], pr[:, 0:FR], AF.Sigmoid)
            rh = hdp.tile([128, FR], bf16, tag="rh")
            nc.vector.tensor_mul(rh[:], rb[:], hd[:])

            # ---- candidate gate, jo-major: ph0 (h-half 0) completes first so
            # tanh(half 0) overlaps the jo 2,3 accumulation
            for jo in range(JT):
                tgt, col = (ph0, jo) if jo < 2 else (ph1, jo - 2)
                for kc in range(KC):
                    nc.tensor.matmul(
                        tgt[:, col * BL:(col + 1) * BL],
                        wht_blk(jo, kc),
                        rh[:, kc * BL:(kc + 1) * BL],
                        start=False, stop=(kc == KC - 1),
                    )
            zf = actp.tile([128, FR], bf16, tag="zf")
            nc.scalar.activation(zf[:], pz[:, 0:FR], AF.Sigmoid)

            # ---- tail fill on PE: half a pair-projection every step
            if t >= 2 and t % 2 == 0:
                pj_cur = pjp.tile([128, PSB], f32, tag="pj")
                issue_proj(t - 2, pj_cur, (0, 1))
            elif t >= 3 and t % 2 == 1:
                issue_proj(t - 3, pj_cur, (2, 3))
            ph0_r, ph1_r = ph0, ph1
            if t + 1 < T:
                pr, pz, ph0, ph1 = make_inits()

            # ---- tanh + blend (h = hd + z*(htl-hd)), then decay for t+1
            hd_n = None
            if t + 1 < T:
                hd_n = hdp.tile([128, FR], bf16, tag="hd")
            for hf, ph in ((0, ph0_r), (1, ph1_r)):
                sl = slice(hf * HB, (hf + 1) * HB)
                htl = actp.tile([128, HB], bf16, tag=f"htl{hf}")
                nc.scalar.activation(htl[:], ph[:, 0:HB], AF.Tanh)
                dd = actp.tile([128, HB], bf16, tag=f"dd{hf}")
                nc.vector.tensor_sub(dd[:], htl[:], hd[:, sl])
                ee = actp.tile([128, HB], bf16, tag=f"ee{hf}")
                nc.vector.tensor_mul(ee[:], zf[:, sl], dd[:])
                nc.vector.tensor_add(ring_half(hf, slot), hd[:, sl], ee[:])
                if t + 1 < T:
                    nc.vector.tensor_mul(
                        hd_n[:, sl],
                        chunks[(t + 1) // GCH][
                            :, ((t + 1) % GCH) * FR + hf * HB:
                               ((t + 1) % GCH) * FR + (hf + 1) * HB],
                        ring_half(hf, slot))
            if t + 1 < T:
                hd = hd_n

            # ---- drain the finished projection pair (psum -> sbuf -> HBM)
            if t >= 3 and t % 2 == 1:
                evac_proj(t - 3, pj_cur)

        # final pair (T-2, T-1)
        pj_cur = pjp.tile([128, PSB], f32, tag="pj")
        issue_proj(T - 2, pj_cur, (0, 1, 2, 3))
        evac_proj(T - 2, pj_cur)

    nc.compile()
    _BUILD_CACHE["nc"] = nc
    return nc


def _host_prep(C, t, Wz, bz, Wr, br, Wh, bh, Wgh, bgh, Wlin):
    """Build per-core input maps (all the precomputed, packed device tensors)."""
    bf = ml_dtypes.bfloat16

    s = Wgh.sum(axis=0)  # (H,)
    t3 = t[:, :, 0]  # (T,B)
    dt = np.concatenate([np.zeros((1, B), np.float32), t3[1:] - t3[:-1]], axis=0)
    # gamma (T,B,H)
    gam = np.exp(-np.maximum(dt[:, :, None] * s[None, None, :] + bgh[None, None, :], 0.0)).astype(np.float32)

    def gate_const(W, b):
        # C @ W_x + colsum(W_m) + b  -> (B,H)
        return C @ W[0:H] + (W[2 * H:3 * H].sum(axis=0) + b)[None, :]

    Az0 = gate_const(Wz, bz).astype(np.float32)
    Ar0 = gate_const(Wr, br).astype(np.float32)
    Ah0 = gate_const(Wh, bh).astype(np.float32)

    Wg = np.stack([Wz[H:2 * H], Wr[H:2 * H]])  # (2,H,H)
    # wzr packed: [k, (kc,g,jo,m)]
    wzr = Wg.reshape(2, KC, 128, JT, 128).transpose(2, 1, 0, 3, 4).reshape(128, KC * 2 * JT * 128)
    wht = Wh[H:2 * H].reshape(KC, 128, JT, 128).transpose(1, 0, 2, 3).reshape(128, KC * JT * 128)
    wlin = Wlin.reshape(KC, 128, O).transpose(1, 0, 2).reshape(128, KC * O)
    wzr = np.ascontiguousarray(wzr, dtype=bf)
    wht = np.ascontiguousarray(wht, dtype=bf)
    wlin = np.ascontiguousarray(wlin, dtype=bf)
    ident = np.eye(128, dtype=bf)

    in_maps = []
    for i in range(NCORES):
        sl = slice(i * BL, (i + 1) * BL)
        gf = gam[:, sl, :]  # (T,BL,H)
        # gam packed: [p, t, kt*BL+b]
        gp = np.ascontiguousarray(
            gf.reshape(T, BL, KC, 128).transpose(3, 0, 2, 1).reshape(128, T, KC * BL),
            dtype=bf)

        def packA(A):
            return np.ascontiguousarray(
                A[sl].reshape(BL, JT, 128).transpose(2, 1, 0).reshape(128, JT * BL), dtype=bf)

        in_maps.append({
            "gam": gp,
            "wzr": wzr,
            "wht": wht,
            "wlin": wlin,
            "a0z": packA(Az0),
            "a0r": packA(Ar0),
            "a0h": packA(Ah0),
            "ident": ident,
        })
    return in_maps


def kernel(C, t, mask, Wz, bz, Wr, br, Wh, bh, Wgh, bgh, wgx, bgx, Wlin, blin,
           _trace=False, _trace_kwargs=None):
    C = np.asarray(C, np.float32)
    t = np.asarray(t, np.float32)
    nc = _build_program()
    in_maps = _host_prep(C, t,
                         np.asarray(Wz, np.float32), np.asarray(bz, np.float32),
                         np.asarray(Wr, np.float32), np.asarray(br, np.float32),
                         np.asarray(Wh, np.float32), np.asarray(bh, np.float32),
                         np.asarray(Wgh, np.float32), np.asarray(bgh, np.float32),
                         np.asarray(Wlin, np.float32))

    from concourse.bass_utils import run_bass_kernel_spmd
    res = run_bass_kernel_spmd(nc, in_maps, list(range(NCORES)),
                               trace=_trace, **(_trace_kwargs or {}))
    outs = [res.results[i]["out"] for i in range(NCORES)]
    full = np.concatenate(outs, axis=1).astype(np.float32)  # (T,B,O)
    full += np.asarray(blin, np.float32)[None, None, :]
    kernel._last_results = res
    return full
